# revision 39
# baseline (speedup 1.0000x reference)
"""BitNet MoE layer (8 experts, top-2, + shared expert) on 8 Trainium2 cores.

Strategy
--------
Expert-parallel: core e owns expert e's FFN over the tokens routed to it
(padded to C), plus a 1/8 data-parallel slice of the shared expert and of the
router logits.

Host side (CPU jax, bit-exact with the fp32 reference for every discontinuous
op): router softmax/top-k (routing decisions), ternary weight quantization
sign(w - mean(w)), and the first-layer rmsnorm + int8 activation quant (the
same xq feeds every expert, so it is computed once).

Device side: all the heavy math. BitNet matmuls are integer-valued
(int8-valued activations x {-1,0,+1} signs), so they run on the PE array in
bf16 with fp32 PSUM accumulation, which is *exact* integer arithmetic (sums
bounded by 2^18 << 2^24) at 4x the fp32 rate. Layer-2 rmsnorm + quant +
round also run on device; rounding uses the +1.5*2^23 magic-number trick
(RNE to integer), matching jnp.round's half-to-even.

Scale bookkeeping: gate/up per-token scales (scale_w / s1_t) are host-built
vectors fused into the Silu/mul drains; the down-proj per-token output scale
is clip(r_t * maxh_t, 1e-4) computed on device, with the remaining scalar
scale_wd/127 folded into the host-side combine weights.
"""

import numpy as np
import ml_dtypes

H = 768
I = 2048
E = 8
T = 8192
SH_SLICE = T // 8  # shared-expert tokens per core
K_H = H // 128  # 6 k-tiles for H contraction
K_I = I // 128  # 16 k-tiles for I contraction
MAGIC = float(np.float32(1.5 * 2.0**23))
BF16 = ml_dtypes.bfloat16
FP8 = ml_dtypes.float8_e4m3fn

_PROGRAM_CACHE = {}

# test-harness hooks (unused in normal operation)
TRACE = False
LAST_RESULTS = None


# --------------------------------------------------------------------------
# device program
# --------------------------------------------------------------------------
def _build_program(C):
    import os
    import concourse.bass as bass
    import concourse.mybir as mybir
    import concourse.tile as tile
    from concourse import bacc
    from contextlib import ExitStack

    F32 = mybir.dt.float32
    B16 = mybir.dt.bfloat16
    F8 = mybir.dt.float8e4
    AF = mybir.ActivationFunctionType
    # CoreSim has no Silu kernel; SIM_TANH=1 swaps in Tanh so the simulator
    # can still execute the identical program structure for race/OOB checks.
    AF_SILU = AF.Tanh if os.environ.get("SIM_TANH") == "1" else AF.Silu
    ALU = mybir.AluOpType
    AX = mybir.AxisListType
    ts = bass.ts

    NT_OWN = C // 128
    NT_SH = SH_SLICE // 128

    nc = bacc.Bacc(None, target_bir_lowering=False)

    # ---- I/O ----
    xq_own = nc.dram_tensor("xq_own", [H, C], B16, kind="ExternalInput")
    xq_sh = nc.dram_tensor("xq_sh", [H, SH_SLICE], B16, kind="ExternalInput")
    mg_own = nc.dram_tensor("mg_own", [128, NT_OWN], F32, kind="ExternalInput")
    mu_own = nc.dram_tensor("mu_own", [128, NT_OWN], F32, kind="ExternalInput")
    mg_sh = nc.dram_tensor("mg_sh", [128, NT_SH], F32, kind="ExternalInput")
    mu_sh = nc.dram_tensor("mu_sh", [128, NT_SH], F32, kind="ExternalInput")
    sg_own = nc.dram_tensor("sg_own", [H, I], F8, kind="ExternalInput")
    su_own = nc.dram_tensor("su_own", [H, I], F8, kind="ExternalInput")
    sd_own = nc.dram_tensor("sd_own", [I, H], F8, kind="ExternalInput")
    sg_sh = nc.dram_tensor("sg_sh", [H, I], F8, kind="ExternalInput")
    su_sh = nc.dram_tensor("su_sh", [H, I], F8, kind="ExternalInput")
    sd_sh = nc.dram_tensor("sd_sh", [I, H], F8, kind="ExternalInput")
    xT = nc.dram_tensor("xT", [H, SH_SLICE], F32, kind="ExternalInput")
    rwT = nc.dram_tensor("rwT", [H, E], F32, kind="ExternalInput")
    rb = nc.dram_tensor("rb", [E, 1], F32, kind="ExternalInput")

    y_own = nc.dram_tensor("y_own", [C, H], F32, kind="ExternalOutput")
    y_sh = nc.dram_tensor("y_sh", [SH_SLICE, H], F32, kind="ExternalOutput")
    lg = nc.dram_tensor("lg", [E, SH_SLICE], F32, kind="ExternalOutput")

    C0 = 1e-4 / float(np.float32(1.0) / np.float32(np.sqrt(np.float32(2e-5))))

    with tile.TileContext(nc) as tc:
        with ExitStack() as ctx:
            const_pool = ctx.enter_context(tc.tile_pool(name="const", bufs=1))
            magic_t = const_pool.tile([128, 1], F32)
            nc.gpsimd.memset(magic_t[:], MAGIC)

            wpool = ctx.enter_context(tc.tile_pool(name="wts", bufs=1))
            xpool = ctx.enter_context(tc.tile_pool(name="xq", bufs=1))
            mpool = ctx.enter_context(tc.tile_pool(name="mv", bufs=1))
            hpool = ctx.enter_context(tc.tile_pool(name="h", bufs=3))
            spool = ctx.enter_context(tc.tile_pool(name="silu", bufs=3))
            scrpool = ctx.enter_context(tc.tile_pool(name="scr", bufs=2))
            q2pool = ctx.enter_context(tc.tile_pool(name="q2", bufs=2))
            qtpool = ctx.enter_context(tc.tile_pool(name="q2T", bufs=3))
            ypool = ctx.enter_context(tc.tile_pool(name="y", bufs=2))
            vpool = ctx.enter_context(tc.tile_pool(name="vec", bufs=30))

            # ---- router logits first: fills PE while expert preloads stream
            with ExitStack() as lctx:
                lpool = lctx.enter_context(tc.tile_pool(name="lgt", bufs=1))
                lpp = lctx.enter_context(
                    tc.tile_pool(name="lg_ps", bufs=2, space="PSUM")
                )
                # router logits (in two half-passes to keep the footprint small)
                x3 = xT.ap().rearrange("(k p) t -> k p t", p=128)
                rw3 = rwT.ap().rearrange("(k p) e -> k p e", p=128)
                rwts = []
                for k in range(K_H):
                    rw_k = lpool.tile([128, E], F32, tag=f"rw{k}", name="rw_k")
                    nc.sync.dma_start(rw_k[:], rw3[k])
                    rwts.append(rw_k)
                rb_t = lpool.tile([E, 1], F32, tag="rb", name="rb_t")
                nc.sync.dma_start(rb_t[:], rb.ap())
                lg_sb = lpool.tile([E, SH_SLICE], F32, tag="lg", name="lg_sb")
                for c in range(SH_SLICE // 512):
                    xts = []
                    for k in range(K_H):
                        xt_k = lpool.tile([128, 512], F32, tag=f"xt{k}", name="xt_k")
                        if c == 0:
                            nc.sync.dma_start(xt_k[:, 0:256], x3[k][:, 0:256])
                            nc.sync.dma_start(
                                xt_k[:, 256:512], x3[k][:, 256:512]
                            )
                        else:
                            nc.sync.dma_start(xt_k[:], x3[k][:, ts(c, 512)])
                        xts.append(xt_k)
                    ps = lpp.tile([E, 512], F32, tag="lgp", name="ps")
                    for k in range(K_H):
                        nc.tensor.matmul(
                                ps[:],
                                rwts[k][:],
                                xts[k][:],
                                start=(k == 0),
                                stop=(k == K_H - 1),
                        )
                    nc.vector.tensor_scalar_add(lg_sb[:, ts(c, 512)], ps[:], rb_t[:])
                nc.sync.dma_start(lg.ap(), lg_sb[:])

            ps1 = ctx.enter_context(tc.tile_pool(name="ps1", bufs=3, space="PSUM"))
            ps2 = ctx.enter_context(tc.tile_pool(name="ps2", bufs=1, space="PSUM"))
            # shared expert first: its small preload lets PE start early while
            # the own-expert preload streams in behind it.
            configs = [
                (xq_own, mg_own, mu_own, sg_own, su_own, sd_own, y_own, NT_OWN),
                (xq_sh, mg_sh, mu_sh, sg_sh, su_sh, sd_sh, y_sh, NT_SH),
            ]
            loaded = []
            for ve, (xq_d, mg_d, mu_d, sg_d, su_d, sd_d, y_d, NT) in enumerate(
                configs
            ):
                NTOK = NT * 128
                xq3 = xq_d.ap().rearrange("(k p) t -> k p t", p=128)
                sg3 = sg_d.ap().rearrange("(k p) i -> k p i", p=128)
                su3 = su_d.ap().rearrange("(k p) i -> k p i", p=128)
                sd3 = sd_d.ap().rearrange("(k p) o -> k p o", p=128)

                mg_t = mpool.tile([128, NT], F32, tag=f"mg{ve}")
                nc.sync.dma_start(mg_t[:], mg_d.ap())
                mu_t = mpool.tile([128, NT], F32, tag=f"mu{ve}")
                nc.sync.dma_start(mu_t[:], mu_d.ap())
                xq_t, sg_t, su_t = [], [], []
                for k in range(K_H):
                    t_ = xpool.tile([128, NTOK], B16, tag=f"xq{ve}_{k}")
                    nc.sync.dma_start(t_[:], xq3[k])
                    xq_t.append(t_)
                    g_ = wpool.tile([128, I], F8, tag=f"sg{ve}_{k}")
                    nc.sync.dma_start(g_[:], sg3[k])
                    sg_t.append(g_)
                    u_ = wpool.tile([128, I], F8, tag=f"su{ve}_{k}")
                    nc.sync.dma_start(u_[:], su3[k])
                    su_t.append(u_)
                sd_t = []
                for k in range(K_I):
                    d_ = wpool.tile([128, H], F8, tag=f"sd{ve}_{k}")
                    nc.sync.dma_start(d_[:], sd3[k])
                    sd_t.append(d_)
                loaded.append((xq_t, sg_t, su_t, sd_t, mg_t, mu_t))

            def expert_tile(
                ti, xq_t, sg_t, su_t, sd_t, mg_t, mu_t, y_d, split_tail=False
            ):
                mg_ap = mg_t[:, ti : ti + 1]
                mu_ap = mu_t[:, ti : ti + 1]
                h_t = hpool.tile([128, I], F32, tag="h", name="h_t")
                mx_p = []
                # ---- layer 1: gate/up matmuls + silu*up drain ----
                for half in range(2):
                    hb = half * 1024
                    pg = ps1.tile([128, 1024], F32, tag="l1", name="pg")
                    pu = ps1.tile([128, 1024], F32, tag="l1", name="pu")
                    for k in range(K_H):
                        lhs = xq_t[k][:, ts(ti, 128)]
                        st = k == 0
                        sp = k == K_H - 1
                        for c in range(2):
                            cs = c * 512
                            nc.tensor.matmul(
                                pg[:, cs : cs + 512], lhs,
                                sg_t[k][:, hb + cs : hb + cs + 512],
                                start=st, stop=sp,
                            )
                            nc.tensor.matmul(
                                pu[:, cs : cs + 512], lhs,
                                su_t[k][:, hb + cs : hb + cs + 512],
                                start=st, stop=sp,
                            )
                    sl = spool.tile([128, 1024], F32, tag="silu", name="sl")
                    nc.scalar.activation(sl[:], pg[:], AF_SILU, scale=mg_ap)
                    nc.vector.scalar_tensor_tensor(
                        out=h_t[:, hb : hb + 1024],
                        in0=pu[:],
                        scalar=mu_ap,
                        in1=sl[:],
                        op0=ALU.mult,
                        op1=ALU.mult,
                    )
                    mx_h = vpool.tile([128, 1], F32, tag="v", name="mx_h")
                    nc.vector.tensor_reduce(
                        out=mx_h[:], in_=h_t[:, hb : hb + 1024], axis=AX.X,
                        op=ALU.max, apply_absolute_value=True,
                    )
                    mx_p.append(mx_h)
                # ---- rounding factor (critical path: only needs maxh) ----
                maxh = vpool.tile([128, 1], F32, tag="v", name="maxh")
                nc.vector.tensor_tensor(
                    out=maxh[:], in0=mx_p[0][:], in1=mx_p[1][:], op=ALU.max
                )
                mc = vpool.tile([128, 1], F32, tag="v", name="mc")
                nc.vector.tensor_scalar_max(mc[:], maxh[:], C0)
                rmc = vpool.tile([128, 1], F32, tag="v", name="rmc")
                nc.vector.reciprocal(rmc[:], mc[:])
                fac = vpool.tile([128, 1], F32, tag="v", name="fac")
                nc.vector.tensor_scalar_mul(fac[:], rmc[:], 127.0)
                # ---- round to int (magic) + bf16 cast + transpose ----
                if not split_tail:
                    t1 = scrpool.tile([128, I], F32, tag="scr", name="t1")
                    nc.scalar.activation(
                        t1[:], h_t[:], AF.Identity, bias=magic_t[:], scale=fac[:]
                    )
                    q2 = q2pool.tile([128, I], B16, tag="q2", name="q2")
                    nc.vector.tensor_scalar_sub(q2[:], t1[:], MAGIC)
                    q2T = qtpool.tile(
                        [128, K_I, 128], B16, tag="q2T", name="q2T"
                    )
                    nc.sync.dma_start_transpose(q2T[:], q2[:])
                    qparts = [q2T]
                else:
                    # final tile: per-half tiles so L2 starts after half one
                    qparts = []
                    for half in range(2):
                        hb = half * 1024
                        t1 = scrpool.tile([128, I // 2], F32, tag="scr", name="t1")
                        nc.scalar.activation(
                            t1[:], h_t[:, hb : hb + 1024], AF.Identity,
                            bias=magic_t[:], scale=fac[:],
                        )
                        q2 = q2pool.tile([128, I // 2], B16, tag="q2", name="q2")
                        nc.vector.tensor_scalar_sub(q2[:], t1[:], MAGIC)
                        qT = qtpool.tile(
                            [128, K_I // 2, 128], B16, tag="q2T", name="qT"
                        )
                        nc.sync.dma_start_transpose(qT[:], q2[:])
                        qparts.append(qT)
                # ---- y output scale uc = clip(r*maxh, 1e-4), off critical path
                sq = scrpool.tile([128, I], F32, tag="scr", name="sq")
                ssq = vpool.tile([128, 1], F32, tag="v", name="ssq")
                nc.scalar.activation(sq[:], h_t[:], AF.Square, accum_out=ssq[:])
                var = vpool.tile([128, 1], F32, tag="v", name="var")
                nc.vector.tensor_scalar(
                    var[:], ssq[:], 1.0 / I, 1e-5, op0=ALU.mult, op1=ALU.max
                )
                varp = vpool.tile([128, 1], F32, tag="v", name="varp")
                nc.vector.tensor_scalar_add(varp[:], var[:], 1e-5)
                rv = vpool.tile([128, 1], F32, tag="v", name="rv")
                nc.vector.reciprocal(rv[:], varp[:])
                r_ = vpool.tile([128, 1], F32, tag="v", name="r_")
                nc.scalar.sqrt(r_[:], rv[:])
                u_ = vpool.tile([128, 1], F32, tag="v", name="u_")
                nc.vector.tensor_mul(u_[:], r_[:], maxh[:])
                uc = vpool.tile([128, 1], F32, tag="v", name="uc")
                nc.vector.tensor_scalar_max(uc[:], u_[:], 1e-4)
                # ---- layer 2 matmul + scaled drain ----
                y_t = ypool.tile([128, H], F32, tag="y", name="y_t")
                pd = ps2.tile([128, 768], F32, tag="l2", name="pd")
                for blk in range(K_I):
                    st = blk == 0
                    sp = blk == K_I - 1
                    lhs = qparts[blk // 8 if split_tail else 0][
                        :, blk % 8 if split_tail else blk, :
                    ]
                    nc.tensor.matmul(
                        pd[:, 0:512], lhs, sd_t[blk][:, 0:512],
                        start=st, stop=sp,
                    )
                    nc.tensor.matmul(
                        pd[:, 512:768], lhs, sd_t[blk][:, 512:768],
                        start=st, stop=sp,
                    )
                nc.scalar.activation(y_t[:], pd[:], AF.Copy, scale=uc[:])
                nc.sync.dma_start(y_d.ap()[ts(ti, 128), :], y_t[:])

            # own-expert tiles
            xq_t, sg_t, su_t, sd_t, mg_t, mu_t = loaded[0]
            for ti in range(NT_OWN):
                expert_tile(ti, xq_t, sg_t, su_t, sd_t, mg_t, mu_t, y_own)

            # shared-expert tiles
            xq_t, sg_t, su_t, sd_t, mg_t, mu_t = loaded[1]
            for ti in range(NT_SH):
                expert_tile(
                    ti, xq_t, sg_t, su_t, sd_t, mg_t, mu_t, y_sh,
                    split_tail=(ti == NT_SH - 1),
                )

    nc.compile()
    return nc


def _get_program(C):
    if C not in _PROGRAM_CACHE:
        _PROGRAM_CACHE[C] = _build_program(C)
    return _PROGRAM_CACHE[C]


# --------------------------------------------------------------------------
# host side
# --------------------------------------------------------------------------
def kernel(
    hidden_states, router_w, router_b, Wg, Wu, Wd, ng, nu, nd,
    sWg, sWu, sWd, sng, snu, snd,
):
    import jax
    import jax.numpy as jnp
    from jax import lax
    from concourse import bass_utils

    cpu = jax.devices("cpu")[0]

    hidden_states = np.asarray(hidden_states, np.float32)
    router_w = np.asarray(router_w, np.float32)
    router_b = np.asarray(router_b, np.float32)
    Wg = np.asarray(Wg, np.float32)
    Wu = np.asarray(Wu, np.float32)
    Wd = np.asarray(Wd, np.float32)
    sWg = np.asarray(sWg, np.float32)
    sWu = np.asarray(sWu, np.float32)
    sWd = np.asarray(sWd, np.float32)
    ng = np.asarray(ng, np.float32)
    nu = np.asarray(nu, np.float32)
    nd = np.asarray(nd, np.float32)
    sng = np.asarray(sng, np.float32)
    snu = np.asarray(snu, np.float32)
    snd = np.asarray(snd, np.float32)

    B, S, _ = hidden_states.shape
    x = hidden_states.reshape(-1, H)

    # The single shared layer-1 quant requires all norm weights to coincide
    # (they are all ones in this problem); nd scales would need a device-side
    # free-dim multiply.
    assert all(np.array_equal(ng[e], sng) for e in range(E))
    assert all(np.array_equal(nu[e], snu) for e in range(E))
    assert np.all(nd == 1.0) and np.all(snd == 1.0)

    EPS = 1e-5

    with jax.default_device(cpu):
        xj = jnp.asarray(x)
        # routing (bit-exact reference ops on CPU)
        logits = jnp.einsum("th,eh->te", xj, jnp.asarray(router_w)) + jnp.asarray(
            router_b
        )
        probs = jax.nn.softmax(logits, axis=-1)
        top_w, top_idx = lax.top_k(probs, 2)
        top_w = top_w / (jnp.sum(top_w, axis=-1, keepdims=True) + 1e-8)
        w_full = jnp.stack(
            [
                jnp.sum(jnp.where(top_idx == e, top_w, 0.0), axis=-1)
                for e in range(E)
            ],
            axis=1,
        )  # [T, E]
        # layer-1 rmsnorm + activation quant (reference ops, norm w == sng)
        xc = jnp.clip(xj, -100.0, 100.0)
        var = jnp.clip(jnp.mean(xc * xc, axis=-1, keepdims=True), EPS, None)
        xn = jnp.clip(xc * lax.rsqrt(var + EPS), -10.0, 10.0) * jnp.asarray(sng)
        xq_c = jnp.clip(xn, -50.0, 50.0)
        mx1 = jnp.clip(jnp.max(jnp.abs(xq_c), axis=-1, keepdims=True), 0.0001, None)
        s1 = 127.0 / mx1
        q1 = jnp.clip(jnp.round(xq_c * s1), -128, 127)

        def wq(w):
            scale = jnp.clip(jnp.mean(jnp.abs(w)), 1e-8, None)
            return jnp.sign(w - jnp.mean(w)), scale

        signs = {}
        scales = {}
        for e in range(E):
            signs[("g", e)], scales[("g", e)] = wq(jnp.asarray(Wg[e]))
            signs[("u", e)], scales[("u", e)] = wq(jnp.asarray(Wu[e]))
            signs[("d", e)], scales[("d", e)] = wq(jnp.asarray(Wd[e]))
        signs["sg"], scales["sg"] = wq(jnp.asarray(sWg))
        signs["su"], scales["su"] = wq(jnp.asarray(sWu))
        signs["sd"], scales["sd"] = wq(jnp.asarray(sWd))

    logits_host = np.asarray(logits)
    top_idx = np.asarray(top_idx)
    w_full = np.asarray(w_full)
    q1 = np.asarray(q1)
    s1 = np.asarray(s1)[:, 0]
    signs = {k: np.asarray(v) for k, v in signs.items()}
    scales = {k: float(v) for k, v in scales.items()}

    # token gather per expert
    idx_e = []
    for e in range(E):
        mask = (top_idx[:, 0] == e) | (top_idx[:, 1] == e)
        idx_e.append(np.nonzero(mask)[0])
    n_e = [len(ix) for ix in idx_e]
    C = max(128, int(-(-max(n_e) // 128)) * 128)
    NT_OWN = C // 128
    NT_SH = SH_SLICE // 128

    q1_bf = q1.astype(BF16)  # exact: small ints
    inv_s1 = (1.0 / s1.astype(np.float64)).astype(np.float64)  # 1/s1 in f64

    def m_vec(scale, tok_idx, pad_to):
        m = (np.float64(scale) * inv_s1[tok_idx]).astype(np.float32)
        out = np.zeros(pad_to, np.float32)
        out[: len(tok_idx)] = m
        return np.ascontiguousarray(out.reshape(-1, 128).T)  # [128, NT]

    rwT = np.ascontiguousarray(router_w.T)
    rb2 = np.ascontiguousarray(router_b.reshape(E, 1))

    sgT_sh = np.ascontiguousarray(signs["sg"].T).astype(FP8)
    suT_sh = np.ascontiguousarray(signs["su"].T).astype(FP8)
    sdT_sh = np.ascontiguousarray(signs["sd"].T).astype(FP8)

    in_maps = []
    for e in range(E):
        ix = idx_e[e]
        xq_own = np.zeros((H, C), BF16)
        xq_own[:, : n_e[e]] = q1_bf[ix].T
        sl = slice(e * SH_SLICE, (e + 1) * SH_SLICE)
        sh_idx = np.arange(e * SH_SLICE, (e + 1) * SH_SLICE)
        in_maps.append(
            {
                "xq_own": xq_own,
                "xq_sh": np.ascontiguousarray(q1_bf[sl].T),
                "mg_own": m_vec(scales[("g", e)], ix, C),
                "mu_own": m_vec(scales[("u", e)], ix, C),
                "mg_sh": m_vec(scales["sg"], sh_idx, SH_SLICE),
                "mu_sh": m_vec(scales["su"], sh_idx, SH_SLICE),
                "sg_own": np.ascontiguousarray(signs[("g", e)].T).astype(FP8),
                "su_own": np.ascontiguousarray(signs[("u", e)].T).astype(FP8),
                "sd_own": np.ascontiguousarray(signs[("d", e)].T).astype(FP8),
                "sg_sh": sgT_sh,
                "su_sh": suT_sh,
                "sd_sh": sdT_sh,
                "xT": np.ascontiguousarray(x[sl].T),
                "rwT": rwT,
                "rb": rb2,
            }
        )

    nc = _get_program(C)
    res = bass_utils.run_bass_kernel_spmd(
        nc, in_maps, core_ids=list(range(8)), trace=TRACE
    )
    global LAST_RESULTS
    LAST_RESULTS = res

    # ---- combine ----
    out = np.zeros((x.shape[0], H), np.float32)
    for e in range(E):
        ix = idx_e[e]
        y = res.results[e]["y_own"][: n_e[e]]
        m = (
            w_full[ix, e].astype(np.float64) * np.float64(scales[("d", e)]) / 127.0
        ).astype(np.float32)
        out[ix] += y * m[:, None]
    m_sh = np.float32(np.float64(scales["sd"]) / 127.0)
    y_shared = np.concatenate([res.results[e]["y_sh"] for e in range(E)], axis=0)
    out += y_shared * m_sh
    out = np.clip(out, -10000.0, 10000.0)

    logits_dev = np.concatenate(
        [res.results[e]["lg"].T for e in range(E)], axis=0
    )
    return out.reshape(B, S, H), logits_dev
